# revision 44
# baseline (speedup 1.0000x reference)
"""DecomposedMaSA Trainium2 kernel (v4).

Full inputs -> shard batch B=8 over 8 NeuronCores (1 image per core) ->
Bass/Tile kernel per core -> gather.

Per-core algorithm (B=1, C=128, H=W=128, heads=4, d=32):
  Host supplies x twice: row-major and spatially-transposed (x2) --
  pure permutations, so VT2 (V column tiles) can be produced just-in-time
  inside the axis-1 pipeline from DMA'd x2 slices (keeps TensorE dense
  and saves SBUF).
  P1: x bf16; Q, K channel-major (scale folded into Wq); VT1 V row tiles.
  Attention per (axis, unit of 4 rows/cols):
    scores: 16 matmuls, u-major so the 4 heads run concurrently in 4
    distinct PSUM banks (two [128,1024] head-pair tiles);
    E = exp(S) on ACT (two ACTIVATEs, PSUM->SBUF bf16, contiguous);
    Et = E * decay on DVE (one [128,2048] bf16 2x op);
    softmax sums+broadcast fused: per head one matmul with all-ones
    [128,32] stationary -> s replicated on the head's partition strip;
    1/s via DVE fast reciprocal; AV via 16 col-tiled matmuls;
    renorm: DVE mul (PSUM x rs -> contiguous bf16 tmp);
      axis1: GPSIMD strided copy tmp -> y_pad columns (keeps the
      pathological strided-dst write off the DVE);
      axis0: DVE in-place 2x-mode add into y_pad rows.
  LePE dwconv3x3 + 1x1 proj fused as 9 shifted matmuls with
  M_tap = w_proj @ diag(w_dw[:, tap]); interleaved into the axis0 phase.
  y_pad geometry: 130 rows x 132 cols, image (r, j) -> y_pad[r+1, j+2]
  (left pad 2 keeps row starts 4B-aligned for DVE 2x adds).
"""

import numpy as np
from contextlib import ExitStack

import concourse.bass as bass
import concourse.tile as tile
from concourse import mybir, bacc
from concourse.bass_utils import run_bass_kernel_spmd

F32 = mybir.dt.float32
BF16 = mybir.dt.bfloat16

C = 128
HW = 128  # H == W == 128
P = 128  # partitions
NHEADS = 4
DHEAD = 32
NPIX = HW * HW  # 16384
PADR = HW + 2  # 130 rows
PADC = HW + 4  # 132 cols
NPAD = PADR * PADC  # 17160

_BUILD_CACHE = {}


def build_kernel(stage="full"):
    nc = bacc.Bacc("TRN2", target_bir_lowering=False, debug=False)

    x_d = nc.dram_tensor("x", [P, NPIX], BF16, kind="ExternalInput")
    x2_d = nc.dram_tensor("x2", [P, NPIX], BF16, kind="ExternalInput")
    wq_d = nc.dram_tensor("wq_t", [P, C], F32, kind="ExternalInput")
    wk_d = nc.dram_tensor("wk_t", [P, C], F32, kind="ExternalInput")
    wv_d = nc.dram_tensor("wv_t", [P, C], F32, kind="ExternalInput")
    d16_d = nc.dram_tensor("d16", [P, 4 * 512], F32, kind="ExternalInput")
    mt_d = nc.dram_tensor("mtaps", [P, 9 * C], F32, kind="ExternalInput")
    out_d = nc.dram_tensor("out", [P, NPIX], F32, kind="ExternalOutput")

    with tile.TileContext(nc) as tc:
        with ExitStack() as ctx:
            _body(ctx, tc, nc, x_d, x2_d, wq_d, wk_d, wv_d, d16_d, mt_d, out_d,
                  stage)
    nc.compile()
    return nc


def _body(ctx, tc, nc, x_d, x2_d, wq_d, wk_d, wv_d, d16_d, mt_d, out_d,
          stage="full"):
    const_pool = ctx.enter_context(tc.tile_pool(name="consts", bufs=1))
    big_pool = ctx.enter_context(tc.tile_pool(name="big", bufs=1))

    # ---- constants in SBUF ----
    wq_sb = const_pool.tile([P, C], BF16, tag="wq")
    wk_sb = const_pool.tile([P, C], BF16, tag="wk")
    wv_sb = const_pool.tile([P, C], BF16, tag="wv")
    d16_sb = const_pool.tile([P, 4 * 512], BF16, tag="d16")
    mt_sb = const_pool.tile([P, 9 * C], BF16, tag="mt")
    ones_sb = const_pool.tile([P, DHEAD], BF16, tag="ones")
    nc.gpsimd.dma_start(wq_sb[:], wq_d[:])
    nc.gpsimd.dma_start(wk_sb[:], wk_d[:])
    nc.gpsimd.dma_start(wv_sb[:], wv_d[:])
    nc.gpsimd.dma_start(d16_sb[:], d16_d[:])
    nc.gpsimd.dma_start(mt_sb[:], mt_d[:])
    nc.gpsimd.memset(ones_sb[:], 1.0)

    # ---- x load (host-pre-cast bf16, no DGE cast); 32 fine chunks so
    # early chunks complete early ----
    x_sb = big_pool.tile([P, NPIX], BF16, tag="x")
    for i in range(32):
        nc.sync.dma_start(
            x_sb[:, i * 512 : (i + 1) * 512], x_d[:, i * 512 : (i + 1) * 512]
        )

    # ---- P1: Q, K channel-major + VT1 (V row tiles) ----
    q_sb = big_pool.tile([P, NPIX], BF16, tag="q")
    k_sb = big_pool.tile([P, NPIX], BF16, tag="k")
    vt1_sb = big_pool.tile([P, NPIX], BF16, tag="vt1")
    ncopy = 0
    # chunk-interleaved emission so compute starts as soon as each DMA'd
    # x chunk lands, and copies spread evenly over ACT/DVE
    with tc.tile_pool(name="proj_ps", bufs=4, space="PSUM") as proj_ps:
        for blk in range(8):
            for w_t, dst in ((wq_sb, q_sb), (wk_sb, k_sb)):
                for ch in range(blk * 4, blk * 4 + 4):
                    ps = proj_ps.tile([P, 512], F32, tag="ps")
                    nc.tensor.matmul(
                        ps[:], w_t[:], x_sb[:, ch * 512 : (ch + 1) * 512]
                    )
                    dsl = dst[:, ch * 512 : (ch + 1) * 512]
                    if ncopy % 2 == 0:
                        nc.scalar.copy(dsl, ps[:])
                    else:
                        nc.vector.tensor_copy(dsl, ps[:])
                    ncopy += 1
            for g in range(blk * 4, blk * 4 + 4):
                ps = proj_ps.tile([P, 512], F32, tag="psv")
                for k in range(4):
                    t = g * 4 + k
                    nc.tensor.matmul(
                        ps[:, k * HW : (k + 1) * HW],
                        x_sb[:, t * HW : (t + 1) * HW],
                        wv_sb[:],
                    )
                dsl = vt1_sb[:, g * 512 : (g + 1) * 512]
                if ncopy % 2 == 0:
                    nc.scalar.copy(dsl, ps[:])
                else:
                    nc.vector.tensor_copy(dsl, ps[:])
                ncopy += 1

    if stage == "proj":
        nc.gpsimd.dma_start(out_d[:, 0:NPIX], q_sb[:])
        return

    # ---- padded output accumulator (reuses x's SBUF slot) ----
    y_pad = big_pool.tile([P, NPAD], BF16, tag="x")
    nc.gpsimd.memset(y_pad[:], 0.0)
    y_pr = y_pad[:].rearrange("p (r c) -> p r c", c=PADC)  # [p, row, col]

    with (
        tc.tile_pool(name="e_pool", bufs=2) as e_pool,
        tc.tile_pool(name="small", bufs=2) as small_pool,
    ):
        # ================= axis 1 (W-axis attention) first =================
        # VT2 column tiles are produced just-in-time from DMA'd x2 slices.
        with (
            tc.tile_pool(name="qk1_ps", bufs=1, space="PSUM") as qk_ps,
            tc.tile_pool(name="stat1_ps", bufs=1, space="PSUM") as stat_ps,
            tc.tile_pool(name="av1_ps", bufs=1, space="PSUM") as av_ps,
            tc.tile_pool(name="vt2_ps", bufs=1, space="PSUM") as vt2_ps,
            tc.tile_pool(name="fil_ps", bufs=1, space="PSUM") as fil_ps,
            tc.tile_pool(name="vt2_ring", bufs=4) as vt2_ring,
            tc.tile_pool(name="x2_ring", bufs=4) as x2_ring,
        ):
            _axis_attention(
                tc, nc, 1, q_sb, k_sb, None, d16_sb, ones_sb, y_pr,
                qk_ps, stat_ps, av_ps, e_pool, small_pool,
                None, None, None, None,
                vt2_work=(vt2_ps, vt2_ring, x2_ring, x2_d, wv_sb),
                fil_ps=fil_ps,
            )

        if stage == "attn1":
            nc.gpsimd.dma_start(out_d[:, 0:NPIX], y_pad[:, 0:NPIX])
            return

        # ========== axis 0 (H-axis attention) + interleaved LePE ==========
        with (
            tc.tile_pool(name="qk0_ps", bufs=1, space="PSUM") as qk_ps,
            tc.tile_pool(name="stat0_ps", bufs=1, space="PSUM") as stat_ps,
            tc.tile_pool(name="av0_ps", bufs=1, space="PSUM") as av_ps,
            tc.tile_pool(name="f_ps", bufs=2, space="PSUM") as f_ps,
            tc.tile_pool(name="o_pool", bufs=3) as o_pool,
        ):
            _axis_attention(
                tc, nc, 0, q_sb, k_sb, vt1_sb, d16_sb, ones_sb, y_pr,
                qk_ps, stat_ps, av_ps, e_pool, small_pool,
                mt_sb, f_ps, o_pool, out_d,
            )


def _axis_attention(tc, nc, axis, q_sb, k_sb, vt_sb, d16_sb, ones_sb, y_pr,
                    qk_ps, stat_ps, av_ps, e_pool, small_pool,
                    mt_sb, f_ps, o_pool, out_d, vt2_work=None, fil_ps=None):
    """One attention axis over 32 units of 4 rows (axis0) / 4 cols (axis1),
    software-pipelined: unit i's scores+exp are emitted one step ahead of
    unit i-1's sums/AV/renorm. axis0 interleaves LePE chunks; axis1
    interleaves just-in-time VT2 production."""
    import os
    sub = os.environ.get("ATTN_SUB", "full")
    Exp = mybir.ActivationFunctionType.Exp
    if axis == 0:
        q_t = q_sb[:].rearrange("p (t j) -> p t j", j=HW)
        k_t = k_sb[:].rearrange("p (t j) -> p t j", j=HW)
    else:
        q_t = q_sb[:].rearrange("p (t j) -> p j t", j=HW)
        k_t = k_sb[:].rearrange("p (t j) -> p j t", j=HW)

    NU = 32
    prev = None
    lepe_done = 0
    vt_tiles = {}  # group -> sbuf tile with VT2 cols [g*512, (g+1)*512)

    def vt2_dma(g):
        vt2_ps, vt2_ring, x2_ring, x2_d, wvf_sb = vt2_work
        xr = x2_ring.tile([P, 512], BF16, tag="x2r")
        nc.sync.dma_start(xr[:], x2_d[:, g * 512 : (g + 1) * 512])
        vt_tiles[("x", g)] = xr

    def vt2_mm(g):
        vt2_ps, vt2_ring, x2_ring, x2_d, wvf_sb = vt2_work
        xr = vt_tiles.pop(("x", g))
        ps = vt2_ps.tile([P, 512], F32, tag="vtp")
        for k in range(4):
            nc.tensor.matmul(ps[:, k * HW : (k + 1) * HW],
                             xr[:, k * HW : (k + 1) * HW], wvf_sb[:])
        vt = vt2_ring.tile([P, 512], BF16, tag="vt2")
        nc.scalar.copy(vt[:], ps[:])
        vt_tiles[g] = vt

    def fillers(n):
        # dependency-free matmuls that keep the PE array busy across
        # cross-engine waits so HAM stays un-throttled (2.4 GHz)
        if fil_ps is None:
            return
        ft = fil_ps.tile([P, 512], F32, tag="fil")
        for _ in range(n):
            nc.tensor.matmul(ft[0:DHEAD, :], ones_sb[:, 0:DHEAD], d16_sb[:, 0:512])

    if vt2_work is not None:
        for g in range(3):
            vt2_dma(g)
        vt2_mm(0)
        vt2_mm(1)
        fillers(20)

    for i in range(NU + 1):
        if i < NU:
            # ---- scores for unit i: two head-pair PSUM tiles, u-major
            # emission so all 4 heads run concurrently in 4 banks ----
            # e layout: cols = h*512 + u*128 + j
            e_sb = e_pool.tile([P, 2048], BF16, tag="e")
            ps_a = qk_ps.tile([P, 1024], F32, tag="qkA")
            ps_b = qk_ps.tile([P, 1024], F32, tag="qkB")
            tiles = [ps_a, ps_b]
            for u in range(4):
                r = i * 4 + u
                for h in range(4):
                    hp = h * DHEAD
                    nc.tensor.matmul(
                        tiles[h // 2][:, (h % 2) * 512 + u * HW : (h % 2) * 512 + (u + 1) * HW],
                        k_t[hp : hp + DHEAD, r, :],
                        q_t[hp : hp + DHEAD, r, :],
                        tile_position=(hp, 0),
                    )
            for half in range(2):
                nc.scalar.activation(
                    e_sb[:, half * 1024 : (half + 1) * 1024], tiles[half][:], Exp
                )
            cur_e = e_sb
            if vt2_work is not None:
                if i + 3 < NU:
                    vt2_dma(i + 3)
                if i + 2 < NU:
                    vt2_mm(i + 2)
            fillers(2)

        if prev is not None and sub != "qkexp":
            (e_p, i_p) = prev
            # ---- decay multiply (DVE, one bf16 2x op) ----
            et_sb = e_pool.tile([P, 2048], BF16, tag="et")
            nc.vector.tensor_mul(et_sb[:], e_p[:], d16_sb[:])

            # ---- softmax sums + broadcast: one matmul per head ----
            ps_stat = stat_ps.tile([P, 512], F32, tag="stat")
            rs_sb = small_pool.tile([P, 512], F32, tag="rs")
            if sub in ("full", "sums"):
                for h in range(4):
                    hp = h * DHEAD
                    nc.tensor.matmul(
                        ps_stat[hp : hp + DHEAD, :],
                        ones_sb[:, 0:DHEAD],
                        e_p[:, h * 512 : (h + 1) * 512],
                        tile_position=(0, hp),
                    )
                nc.vector.reciprocal_approx_fast(rs_sb[:], ps_stat[:])
            else:
                nc.gpsimd.memset(rs_sb[:], 1.0)

            if sub in ("et", "sums"):
                prev = (cur_e, i) if i < NU else None
                continue

            # ---- attention @ V (col-tiled per head) ----
            src_vt = vt_tiles.pop(i_p) if vt2_work is not None else vt_sb
            ps_av = av_ps.tile([P, 512], F32, tag="av")
            for u in range(4):
                for h in range(4):
                    hp = h * DHEAD
                    if vt2_work is not None:
                        lhsT = src_vt[:, u * HW + hp : u * HW + hp + DHEAD]
                    else:
                        r = i_p * 4 + u
                        lhsT = vt_sb[:, r * HW + hp : r * HW + hp + DHEAD]
                    nc.tensor.matmul(
                        ps_av[hp : hp + DHEAD, u * HW : (u + 1) * HW],
                        lhsT,
                        et_sb[:, h * 512 + u * HW : h * 512 + (u + 1) * HW],
                        tile_position=(0, hp),
                    )

            if axis == 1:
                fillers(2)
            # ---- renorm + write into padded y ----
            tmp = small_pool.tile([P, 512], BF16, tag="tmp")
            nc.vector.tensor_mul(tmp[:], ps_av[:], rs_sb[:])
            if axis == 1:
                # y_pad[:, (z+1)*132 + (4*i_p+2+u)] <- tmp[u, z]
                # strided-inner-dst write on GPSIMD (DVE is the bottleneck)
                tmp_zu = tmp[:].rearrange("p (u z) -> p z u", z=HW)
                dst = y_pr[:, 1 : HW + 1, i_p * 4 + 2 : i_p * 4 + 6]
                nc.gpsimd.tensor_copy(dst, tmp_zu)
            else:
                dst = y_pr[:, i_p * 4 + 1 : i_p * 4 + 5, 2 : HW + 2]
                nc.vector.tensor_add(
                    dst, dst, tmp[:].rearrange("p (u j) -> p u j", j=HW)
                )

        # ---- interleaved LePE (axis0 only) ----
        if axis == 0 and mt_sb is not None and sub == "full":
            while lepe_done < NU and lepe_done + 2 <= i - 1:
                _lepe_chunk(nc, f_ps, o_pool, mt_sb, y_pr, out_d, lepe_done)
                lepe_done += 1

        prev = (cur_e, i) if i < NU else None

    if axis == 0 and mt_sb is not None and sub == "full":
        while lepe_done < NU:
            _lepe_chunk(nc, f_ps, o_pool, mt_sb, y_pr, out_d, lepe_done)
            lepe_done += 1


def _lepe_chunk(nc, f_ps, o_pool, mt_sb, y_pr, out_d, c):
    """Fused dwconv3x3 + 1x1 proj for 4 image rows starting at 4c."""
    r0 = c * 4
    ps_f = f_ps.tile([P, 512], F32, tag="f")
    t = 0
    for dy in range(3):
        for dx in range(3):
            nc.tensor.matmul(
                ps_f[:],
                mt_sb[:, t * C : (t + 1) * C],
                y_pr[:, r0 + dy : r0 + dy + 4, 1 + dx : 1 + dx + HW],
                start=(t == 0),
                stop=(t == 8),
            )
            t += 1
    o_sb = o_pool.tile([P, 512], F32, tag="o")
    nc.scalar.copy(o_sb[:], ps_f[:])
    nc.sync.dma_start(out_d[:, r0 * HW : (r0 + 4) * HW], o_sb[:])


def _host_prep(x, w_qkv, w_proj, w_dw, gamma):
    B = x.shape[0]
    scale = DHEAD ** -0.5
    wq_t = np.ascontiguousarray((w_qkv[0:C] * scale).T).astype(np.float32)
    wk_t = np.ascontiguousarray(w_qkv[C : 2 * C].T).astype(np.float32)
    wv_t = np.ascontiguousarray(w_qkv[2 * C : 3 * C].T).astype(np.float32)
    idx = np.arange(HW, dtype=np.float32)
    dmat = np.exp(-float(gamma) * np.abs(idx[:, None] - idx[None, :])).astype(np.float32)
    d16 = np.tile(np.concatenate([dmat] * 4, axis=1), (1, 4))  # [128, 2048]
    # M_tap = w_proj @ diag(w_dw[:, 0, dy, dx]); lhsT layout -> M_tap.T [c, o]
    mtaps = np.empty((P, 9 * C), dtype=np.float32)
    t = 0
    for dy in range(3):
        for dx in range(3):
            m = w_proj * w_dw[:, 0, dy, dx][None, :]  # [o, c] * diag over c
            mtaps[:, t * C : (t + 1) * C] = m.T
            t += 1
    import ml_dtypes
    xs = x.reshape(B, C, NPIX).astype(ml_dtypes.bfloat16)
    # spatially-transposed copy: xs2[b][c, t*128+z] = x[b, c, z, t]
    xs2 = np.ascontiguousarray(x.transpose(0, 1, 3, 2)).reshape(B, C, NPIX)
    xs2 = xs2.astype(ml_dtypes.bfloat16)
    return xs, xs2, wq_t, wk_t, wv_t, d16.astype(np.float32), mtaps


def kernel(x, w_qkv, w_proj, w_dw, gamma):
    x = np.asarray(x, dtype=np.float32)
    w_qkv = np.asarray(w_qkv, dtype=np.float32)
    w_proj = np.asarray(w_proj, dtype=np.float32)
    w_dw = np.asarray(w_dw, dtype=np.float32)
    gamma = np.float32(gamma)

    B = x.shape[0]
    xs, xs2, wq_t, wk_t, wv_t, d16, mtaps = _host_prep(x, w_qkv, w_proj, w_dw, gamma)

    if "nc" not in _BUILD_CACHE:
        _BUILD_CACHE["nc"] = build_kernel()
    nc = _BUILD_CACHE["nc"]

    in_maps = [
        {
            "x": np.ascontiguousarray(xs[b]),
            "x2": np.ascontiguousarray(xs2[b]),
            "wq_t": wq_t,
            "wk_t": wk_t,
            "wv_t": wv_t,
            "d16": d16,
            "mtaps": mtaps,
        }
        for b in range(B)
    ]
    res = run_bass_kernel_spmd(nc, in_maps, core_ids=list(range(8)))
    out = np.stack([res.results[b]["out"].reshape(C, HW, HW) for b in range(B)])
    return out.astype(np.float32)
